# revision 6
# baseline (speedup 1.0000x reference)
"""Head-parallel multi-head attention on 8 Trainium2 NeuronCores.

Sharding: 2 heads per core (head axis split across 8 cores). Each core
computes its heads' Q/K/V projections (block-diagonal 128x128 weights,
both heads packed), full attention for its 2 heads, and a partial W_o
projection over its 128 head-dims. The host sums the 8 partial outputs
(the "all-gather + W_o" is algebraically a sum of per-core partial
matmuls) and adds b_o.

On-chip layout (per core, per batch b):
  xT      [128, 2048]  x slice transposed (pair dims on partitions)
  Q^T,K^T [128, 2048]  pair-stacked projections (head0 rows 0-63)
  S^T     [128k, 512q] scores transposed, per 128-row k-tile (PSUM)
  exp     ACT exp(0.125*S) PSUM->SBUF, bf16 slab [k-tile, q-chunk]
  PV      out[d+1, q] = [V | 1]^T @ expS^T  (row 64 = softmax denom)
  norm    DVE recip(denom) -> TensorE K=1 broadcast -> DVE multiply
  out     per-head accumulated W_o partial matmul -> DRAM

Matmuls run in float32r (full-rate fp32, ~tf32-ish rounding); inputs are
converted fp32->fp32r by the producing DMA or DVE op. The PV matmul and
its exp'd-scores operand are bf16.
"""

import os
import sys
from contextlib import ExitStack

import numpy as np

for _p in ("/opt/trn_rl_repo", os.path.expanduser("~/.axon_site/_ro/trn_rl_repo")):
    if os.path.isdir(_p) and _p not in sys.path:
        sys.path.append(_p)

import concourse.bass as bass
import concourse.tile as tile
from concourse import mybir
from concourse.bass_utils import run_bass_kernel_spmd

B, S, E, H = 2, 2048, 1024, 16
Dh = E // H           # 64
NCORES = 8
HPC = H // NCORES     # 2 heads per core
PD = HPC * Dh         # 128 pair dims per core
QC = 512              # q-chunk width
NQC = S // QC         # 4
KT = 128              # k-tile rows
NKT = S // KT         # 16
EC = 512              # e-chunk in out projection
F32 = mybir.dt.float32
F32R = mybir.dt.float32r
BF16 = mybir.dt.bfloat16
EXP = mybir.ActivationFunctionType.Exp


def split_multi_waits(nc):
    """Split multi-wait instructions into chained single-wait EventSemaphores.

    The walrus build here accepts at most ONE sync-wait command per
    instruction, while Tile emits several. Rewrite each instruction with
    N>1 waits into (N-1) same-engine EventSemaphore instructions (one
    wait each) followed by the instruction keeping its last wait —
    per-engine program order makes this equivalent.
    """
    n_split = 0
    for f in nc.m.functions:
        for blk in f.blocks:
            insts = list(blk.instructions)
            new = []
            for inst in insts:
                si = inst.sync_info
                waits = list(si.on_wait) if si is not None and si.on_wait else []
                if len(waits) > 1:
                    for j, w in enumerate(waits[:-1]):
                        ev = mybir.InstEventSemaphore(
                            name=f"{inst.name}-wsplit{j}", ins=[], outs=[]
                        )
                        ev.engine = inst.engine
                        ev.sync_info = mybir.SyncInfo(on_wait=[w], on_update=[])
                        nc.register_instruction(ev, overwrite=True)
                        new.append(ev)
                    si.on_wait = waits[-1:]
                    n_split += 1
                new.append(inst)
            blk.instructions = new
    return n_split


def build_program():
    nc = bass.Bass("TRN2", target_bir_lowering=False, debug=False)

    xT = nc.dram_tensor("xT", [B, PD, S], F32, kind="ExternalInput").ap()
    wqkv = nc.dram_tensor("wqkv", [3, PD, PD], F32, kind="ExternalInput").ap()
    bqk = nc.dram_tensor("bqk", [2, PD, 1], F32, kind="ExternalInput").ap()
    bvb = nc.dram_tensor("bvb", [PD, PD], F32, kind="ExternalInput").ap()
    wo = nc.dram_tensor("wo", [HPC, Dh, E], F32, kind="ExternalInput").ap()
    ones = nc.dram_tensor("ones", [1, Dh], F32, kind="ExternalInput").ap()
    out = nc.dram_tensor("out", [B, S, E], F32, kind="ExternalOutput").ap()

    with tile.TileContext(nc) as tc, ExitStack() as ctx:
        const = ctx.enter_context(tc.tile_pool(name="const", bufs=1))
        perb = ctx.enter_context(tc.tile_pool(name="perb", bufs=1))
        slabp = ctx.enter_context(tc.tile_pool(name="slab", bufs=10))
        stage = ctx.enter_context(tc.tile_pool(name="stage", bufs=4))
        small = ctx.enter_context(tc.tile_pool(name="small", bufs=4))
        psc = ctx.enter_context(tc.tile_pool(name="psc", bufs=1, space="PSUM"))
        poa = ctx.enter_context(tc.tile_pool(name="poa", bufs=1, space="PSUM"))
        pmisc = ctx.enter_context(tc.tile_pool(name="pmisc", bufs=2, space="PSUM"))

        # --- load constants (DMA converts fp32 -> fp32r where needed) ---
        xt_sb = const.tile([PD, B, S], F32R)
        for b in range(B):
            nc.gpsimd.dma_start(out=xt_sb[:, b, :], in_=xT[b])
        w_sb = []
        for i in range(3):
            w = const.tile([PD, PD], F32R, tag=f"w{i}", name=f"w{i}")
            nc.gpsimd.dma_start(out=w[:], in_=wqkv[i])
            w_sb.append(w)
        bq_sb = const.tile([PD, 1], F32, tag="bq")
        nc.sync.dma_start(out=bq_sb[:], in_=bqk[0])
        bk_sb = const.tile([PD, 1], F32, tag="bk")
        nc.sync.dma_start(out=bk_sb[:], in_=bqk[1])
        bvb_sb = const.tile([PD, PD], F32, tag="bvb")
        nc.sync.dma_start(out=bvb_sb[:], in_=bvb)
        wo_sb = []
        for h in range(HPC):
            t = const.tile([Dh, E], F32R, tag=f"wo{h}", name=f"wo{h}")
            nc.gpsimd.dma_start(out=t[:], in_=wo[h])
            wo_sb.append(t)
        ones_sb = const.tile([1, Dh], F32, tag="ones")
        nc.sync.dma_start(out=ones_sb[:], in_=ones)

        for b in range(B):
            # --- Q^T / K^T projections (pair-stacked, [o, s] layout) ---
            qt = perb.tile([PD, S], F32R, tag="qt")
            kt_t = perb.tile([PD, S], F32R, tag="kt")
            for j in range(NQC):
                sl_ = slice(j * QC, (j + 1) * QC)
                mq = pmisc.tile([PD, QC], F32, tag="mm")
                nc.tensor.matmul(mq[:], lhsT=w_sb[0][:], rhs=xt_sb[:, b, sl_])
                nc.vector.tensor_scalar_add(qt[:, sl_], mq[:], bq_sb[:])
                mk = pmisc.tile([PD, QC], F32, tag="mm")
                nc.tensor.matmul(mk[:], lhsT=w_sb[1][:], rhs=xt_sb[:, b, sl_])
                nc.vector.tensor_scalar_add(kt_t[:, sl_], mk[:], bk_sb[:])

            # --- V projection, natural [s, d] layout, bf16, with ones col ---
            # vaug[:, st, h, 0:64] = V rows; vaug[:, st, h, 64] = 1.0
            vaug = perb.tile([PD, NKT, HPC, Dh + 1], BF16, tag="vaug")
            nc.vector.memset(vaug[:, :, :, Dh], 1.0)
            for st in range(NKT):
                ssl = slice(st * KT, (st + 1) * KT)
                mv = pmisc.tile([PD, PD], F32, tag="mm")
                nc.tensor.matmul(mv[:], lhsT=xt_sb[:, b, ssl], rhs=w_sb[2][:])
                nc.vector.tensor_add(
                    vaug[:, st, :, 0:Dh],
                    mv[:].rearrange("p (t d) -> p t d", t=HPC),
                    bvb_sb[:].rearrange("p (t d) -> p t d", t=HPC),
                )

            ot = [
                perb.tile([Dh, S], F32R, tag=f"ot{h}", name=f"ot{h}")
                for h in range(HPC)
            ]

            for qc in range(NQC):
                qsl = slice(qc * QC, (qc + 1) * QC)
                slabs = [[], []]
                # --- scores (transposed) + exp, two k-tiles per round ---
                for r8 in range(NKT // 2):
                    for h in range(HPC):
                        hsl = slice(Dh * h, Dh * (h + 1))
                        sc = psc.tile([PD, 2 * QC], F32, tag=f"sc{h}")
                        for j in range(2):
                            kti = 2 * r8 + j
                            nc.tensor.matmul(
                                sc[:, j * QC:(j + 1) * QC],
                                lhsT=kt_t[hsl, kti * KT:(kti + 1) * KT],
                                rhs=qt[hsl, qsl],
                            )
                        sl_t = slabp.tile([PD, 2 * QC], BF16, tag=f"sl{h}")
                        nc.scalar.activation(sl_t[:], sc[:], EXP, scale=0.125)
                        slabs[h].append(sl_t)

                # --- PV + denominator + normalize ---
                for h in range(HPC):
                    oa = poa.tile([Dh + 1, QC], F32, tag=f"oa{h}")
                    for r8 in range(NKT // 2):
                        for j in range(2):
                            nc.tensor.matmul(
                                oa[:],
                                lhsT=vaug[:, 2 * r8 + j, h, :],
                                rhs=slabs[h][r8][:, j * QC:(j + 1) * QC],
                                start=(r8 == 0 and j == 0),
                                stop=(r8 == NKT // 2 - 1 and j == 1),
                            )
                    rr = small.tile([1, QC], F32, tag="rr")
                    nc.vector.reciprocal(rr[:], oa[Dh:Dh + 1, :])
                    # replicate 1/denom across 64 partitions: K=1 matmul
                    # against a ones row, then stage through SBUF (the
                    # normalize multiply may read only one PSUM operand)
                    bc = pmisc.tile([Dh, QC], F32, tag="mm")
                    nc.tensor.matmul(bc[:], lhsT=ones_sb[:], rhs=rr[:])
                    bcs = small.tile([Dh, QC], F32, tag="bcs")
                    nc.vector.tensor_copy(bcs[:], bc[:])
                    nc.vector.tensor_mul(ot[h][:, qsl], oa[0:Dh, :], bcs[:])

                # --- output projection for this q-chunk's s-tiles ---
                for st in range(qc * (QC // KT), (qc + 1) * (QC // KT)):
                    ssl = slice(st * KT, (st + 1) * KT)
                    outsb = stage.tile([KT, E], F32)
                    for e in range(E // EC):
                        esl = slice(e * EC, (e + 1) * EC)
                        po = pmisc.tile([KT, EC], F32, tag="mm")
                        nc.tensor.matmul(po[:], lhsT=ot[0][:, ssl],
                                         rhs=wo_sb[0][:, esl],
                                         start=True, stop=False)
                        nc.tensor.matmul(po[:], lhsT=ot[1][:, ssl],
                                         rhs=wo_sb[1][:, esl],
                                         start=False, stop=True)
                        nc.vector.tensor_copy(outsb[:, esl], po[:])
                    nc.sync.dma_start(out=out[b, ssl, :], in_=outsb[:])

    split_multi_waits(nc)
    return nc


def prep_core_inputs(c, x, Wq, Wk, Wv, bq, bk, bv, Wo):
    h0, h1 = HPC * c, HPC * c + 1
    xT_c = np.ascontiguousarray(
        np.transpose(x[:, :, c * PD:(c + 1) * PD], (0, 2, 1))
    ).astype(np.float32)
    wqkv = np.zeros((3, PD, PD), np.float32)
    for i, W in enumerate((Wq, Wk, Wv)):
        wqkv[i, :Dh, :Dh] = W[h0]
        wqkv[i, Dh:, Dh:] = W[h1]
    bqk = np.stack([
        np.concatenate([bq[h0], bq[h1]])[:, None],
        np.concatenate([bk[h0], bk[h1]])[:, None],
    ]).astype(np.float32)
    bvb = np.tile(np.concatenate([bv[h0], bv[h1]])[None, :], (PD, 1)).astype(np.float32)
    wo_c = np.stack([Wo[h0 * Dh:(h0 + 1) * Dh], Wo[h1 * Dh:(h1 + 1) * Dh]]).astype(np.float32)
    return {
        "xT": xT_c,
        "wqkv": wqkv,
        "bqk": bqk,
        "bvb": bvb,
        "wo": wo_c,
        "ones": np.ones((1, Dh), np.float32),
    }


_CACHE = {}


def _get_nc():
    if "nc" not in _CACHE:
        _CACHE["nc"] = build_program()
    return _CACHE["nc"]


def kernel(x, Wq, Wk, Wv, bq, bk, bv, Wo, bo, _trace=False, _trace_kwargs=None):
    x, Wq, Wk, Wv, bq, bk, bv, Wo, bo = (
        np.asarray(a, np.float32) for a in (x, Wq, Wk, Wv, bq, bk, bv, Wo, bo)
    )
    nc = _get_nc()
    in_maps = [
        prep_core_inputs(c, x, Wq, Wk, Wv, bq, bk, bv, Wo) for c in range(NCORES)
    ]
    res = run_bass_kernel_spmd(
        nc, in_maps, list(range(NCORES)), trace=_trace, **(_trace_kwargs or {})
    )
    acc = res.results[0]["out"].copy()
    for c in range(1, NCORES):
        acc += res.results[c]["out"]
    acc += bo[None, None, :]
    if _trace:
        _CACHE["last_results"] = res
    return acc
